# revision 10
# baseline (speedup 1.0000x reference)
"""DeepSeek-style GQA attention block (B=2, S=2048, H=1536, 12 q-heads /
2 kv-heads, d=128) sharded over 8 TRN2 NeuronCores.

Sharding: core = (batch b, kv-group hh, query-half th).
  - tensor parallel over the 2 kv groups (6 q-heads + 1 kv head each)
  - data parallel over batch (2)
  - query-token parallel (2 halves of 1024)
Each core computes its 6 heads' attention for its 1024 query tokens against
the full 2048-token K/V of its kv head, then a partial O-projection; the two
kv-group partials per (b, th) are summed on the host.

All matmuls run in bf16 with fp32 PSUM accumulation. Softmax runs without
max-subtraction (scores are O(1) here), with the 1/sqrt(d) scale and the
additive attention-mask bias fused into the ACT exp instruction.

Layout trick: scores are computed TRANSPOSED (scores^T[Sk, Sq] = K Q^T) so
the attention probabilities come out with Sk on partitions, which is exactly
the layout the AV matmul needs as its moving operand.

Softmax denominator: exp chunks are accumulated on the DVE (bf16 adds, with
the last chunk folded directly into the reduction matmul), then ONE ones-
vector matmul per head-half does the 128-partition reduction, a rank-1
matmul broadcasts it back over the d partitions, and DVE reciprocal/multiply
normalizes the AV output. This costs the PE ~1.3us/head instead of the
~8us/head of a per-chunk rowsum scheme, and keeps the slow GpSimd engine
(partition_all_reduce measured 6.7us!) off the critical path entirely.

Scheduling: TRN2's PE p-state throttle only reaches 2.4 GHz after 3us of
uninterrupted work, so the kernel keeps the PE queue dense. All deferrable
matmul work — Q-projection for heads 1-5 and the first two O-projection
passes (h0-2, h3-4, merged through a bf16 SBUF partial) — is enqueued as
closures and pumped one unit per chunk-iteration inside the attention
loops, filling what would otherwise be exp-wait bubbles. After head 5 only
a single-matmul O-pass remains: partial + h5 contribution accumulate in
PSUM via an identity matmul, with writebacks alternating ACT/DVE.
"""

import numpy as np
import ml_dtypes

HIDDEN = 1536
D = 128          # head dim
NH = 6           # q-heads per core
B, S = 2, 2048
SQ = 1024        # query tokens per core
HC = HIDDEN // 128   # 12 hidden chunks
SKC = S // 128       # 16 key chunks
SCALE = float(1.0 / np.sqrt(np.float32(D)))

_NC_CACHE = {}
last_results = None  # BassKernelResults of the most recent run (for test.py)


def _build_nc():
    import concourse.bacc as bacc
    import concourse.mybir as mybir
    import concourse.tile as tile
    from concourse.masks import make_identity

    bf16 = mybir.dt.bfloat16
    f32 = mybir.dt.float32
    Exp = mybir.ActivationFunctionType.Exp
    Copy = mybir.ActivationFunctionType.Copy

    nc = bacc.Bacc("TRN2", target_bir_lowering=False, debug=False, num_devices=8)

    # All inputs are pre-rearranged on the host into [128-partition, ...]
    # layouts so every DMA line is a large contiguous run (few descriptors).
    # xt holds only this core's 1024 query tokens: K/V for the other half
    # comes from the partner core via a pairwise AllGather.
    xt = nc.dram_tensor("xt", [128, HC, SQ], bf16, kind="ExternalInput")
    wqt = nc.dram_tensor("wqt", [128, HC, NH * D], bf16, kind="ExternalInput")
    wkt = nc.dram_tensor("wkt", [128, HC, D], bf16, kind="ExternalInput")
    wvt = nc.dram_tensor("wvt", [128, HC, D], bf16, kind="ExternalInput")
    wot = nc.dram_tensor("wot", [128, NH, HIDDEN], bf16, kind="ExternalInput")
    biasd = nc.dram_tensor("biasd", [128, SKC], f32, kind="ExternalInput")
    y = nc.dram_tensor("y", [SQ, HIDDEN], bf16, kind="ExternalOutput")

    NB = HIDDEN // 512   # 3 o-proj column blocks
    NT = SQ // 128       # 8 o-proj token blocks
    NU = NT * NB         # 24 o-proj tiles

    with tile.TileContext(nc) as tc:
        with (
            tc.tile_pool(name="const", bufs=1) as constp,
            tc.tile_pool(name="weights", bufs=1) as wp,
            tc.tile_pool(name="persist", bufs=1) as pers,
        ):
            ident = constp.tile([128, 128], bf16)
            make_identity(nc, ident[:])
            ones_col = constp.tile([128, 1], bf16)
            nc.vector.memset(ones_col[:], 1.0)
            ones_row = constp.tile([1, 128], bf16)
            nc.vector.memset(ones_row[:], 1.0)
            bias_sb = constp.tile([128, SKC], f32)

            wk_sb = wp.tile([128, HC, D], bf16)
            wv_sb = wp.tile([128, HC, D], bf16)
            wo_sb = wp.tile([128, NH, HIDDEN], bf16)

            kT_sb = pers.tile([128, S], bf16)         # K^T [d, Sk] (global order)
            v_sb = pers.tile([128, SKC, D], bf16)     # V [Sk, d], chunked
            qT_sb = pers.tile([128, NH, SQ], bf16)    # Q^T [d, Sq] per head
            outT_sb = pers.tile([128, NH, SQ], bf16)  # normalized AV out^T
            part_sb = pers.tile([128, NU, 512], bf16) # o-proj partials
            xt_sb = pers.tile([128, HC, SQ], bf16)
            wq_sb = pers.tile([128, HC, NH * D], bf16)
            vT_sb = pers.tile([128, SQ], bf16)
            ccs_sb = pers.tile([128, 2048], bf16)     # [kT half | v half] staging

            def dma_psplit(dst, src, n):
                # split a [128, ...] DMA into n partition ranges so the
                # per-partition-line descriptors issue on n queues in parallel
                step = 128 // n
                for i in range(n):
                    sl = slice(step * i, step * (i + 1))
                    nc.sync.dma_start(dst[sl], src[sl])

            # DMA order is the early-pipeline schedule. Each transfer moves
            # whole-tensor per-partition contiguous lines (descriptor cost is
            # ~constant per line, so bigger lines = more GB/s per queue),
            # partition-split across queues for parallelism. wo is issued
            # LAST (after the collective's bounce traffic) since o-proj
            # weights aren't needed until after head 2.
            dma_psplit(wk_sb[:], wkt.ap(), 2)
            dma_psplit(wv_sb[:], wvt.ap(), 2)
            dma_psplit(xt_sb[:], xt.ap(), 16)
            dma_psplit(wq_sb[:], wqt.ap(), 8)
            nc.sync.dma_start(bias_sb[:], biasd.ap())

            # ---------- local K/V half + exchange, Q0 projection ----------
            # Each core projects K/V only for its own 1024 tokens; the two
            # query-half cores of the same (batch, kv-group) swap halves via
            # a pairwise AllGather through DRAM bounce buffers. kT_sb / v_sb
            # end up in GLOBAL token order on both cores.
            with (
                tc.tile_pool(name="kv_ps", bufs=1, space="PSUM") as kvps,
                tc.tile_pool(name="proj_ps", bufs=1, space="PSUM") as pps,
                tc.tile_pool(name="vtr_ps", bufs=2, space="PSUM") as vtrp,
                tc.tile_pool(name="cc_dram", bufs=1, space="DRAM") as ccd,
            ):
                ccin = ccd.tile([128, 2048], bf16)
                ccout = ccd.tile([2, 128, 2048], bf16)
                qps = pps.tile([128, SQ], f32, tag="projq")

                kv = kvps.tile([128, 2, 2, 512], f32, tag="kv")
                for c in range(HC):
                    for ti, w_sb in ((0, wk_sb), (1, wv_sb)):
                        for sb in range(2):
                            nc.tensor.matmul(
                                kv[:, ti, sb, :],
                                w_sb[:, c, :],
                                xt_sb[:, c, 512 * sb : 512 * (sb + 1)],
                                start=(c == 0),
                                stop=(c == HC - 1),
                            )
                for sb in range(2):
                    nc.vector.tensor_copy(
                        vT_sb[:, 512 * sb : 512 * (sb + 1)], kv[:, 1, sb, :]
                    )
                for sb in range(2):
                    nc.vector.tensor_copy(
                        ccs_sb[:, 512 * sb : 512 * (sb + 1)], kv[:, 0, sb, :]
                    )
                for c in range(8):
                    pt = vtrp.tile([128, 128], bf16, tag="vtr")
                    nc.tensor.transpose(
                        pt[:], vT_sb[:, 128 * c : 128 * (c + 1)], ident[:]
                    )
                    nc.vector.tensor_copy(
                        ccs_sb[:, 1024 + 128 * c : 1024 + 128 * (c + 1)], pt[:]
                    )
                dma_psplit(ccin[:, :], ccs_sb[:, :], 4)
                nc.gpsimd.collective_compute(
                    "AllGather",
                    mybir.AluOpType.bypass,
                    replica_groups=[[0, 1], [2, 3], [4, 5], [6, 7]],
                    ins=[ccin.opt()],
                    outs=[ccout.opt()],
                )
                for sqh in range(2):
                    for c in range(HC):
                        nc.tensor.matmul(
                            qps[:, 512 * sqh : 512 * (sqh + 1)],
                            wq_sb[:, c, 0:D],
                            xt_sb[:, c, 512 * sqh : 512 * (sqh + 1)],
                            start=(c == 0),
                            stop=(c == HC - 1),
                        )
                nc.vector.tensor_copy(qT_sb[:, 0, :], qps[:])
                for th in range(2):
                    dma_psplit(
                        kT_sb[:, 1024 * th : 1024 * (th + 1)],
                        ccout[th, :, 0:1024], 2,
                    )
                    dma_psplit(
                        v_sb[:, 8 * th : 8 * (th + 1), :],
                        ccout[th, :, 1024:2048], 2,
                    )
                dma_psplit(wo_sb[:], wot.ap(), 8)

            # ---------- attention + pipelined projections ----------
            with (
                tc.tile_pool(name="sc_ps", bufs=2, space="PSUM") as scp,
                tc.tile_pool(name="av_ps", bufs=1, space="PSUM") as avp,
                tc.tile_pool(name="y_ps", bufs=2, space="PSUM") as yp,
                tc.tile_pool(name="esb", bufs=8) as ep,
                tc.tile_pool(name="eaccp", bufs=2) as eaccp,
                tc.tile_pool(name="rowp", bufs=4) as rowp,
                tc.tile_pool(name="brecp", bufs=2) as brecp,
                tc.tile_pool(name="avsb", bufs=2) as avsbp,
                tc.tile_pool(name="y_sb", bufs=6) as ysb,
            ):
                # deferred PE work: ('q', head, fn) or ('o', fn)
                unit_q = []

                def pump(n):
                    for _ in range(min(n, len(unit_q))):
                        unit_q.pop(0)[-1]()

                def drain_q(h):
                    while unit_q and unit_q[0][0] == "q" and unit_q[0][1] <= h:
                        unit_q.pop(0)[-1]()

                def q_subunit(qh, sqh, cg, cell):
                    # 3-matmul slice of one Q head-half projection; the four
                    # slices share a PSUM tile carried through `cell`
                    def run():
                        if cg == 0:
                            cell["ps"] = yp.tile(
                                [128, 512], f32, tag="y", name=f"qp{qh}_{sqh}"
                            )
                        ps = cell["ps"]
                        for c in range(3 * cg, 3 * cg + 3):
                            nc.tensor.matmul(
                                ps[:],
                                wq_sb[:, c, D * qh : D * (qh + 1)],
                                xt_sb[:, c, 512 * sqh : 512 * (sqh + 1)],
                                start=(c == 0),
                                stop=(c == HC - 1),
                            )
                        if cg == 3:
                            nc.vector.tensor_copy(
                                qT_sb[:, qh, 512 * sqh : 512 * (sqh + 1)], ps[:]
                            )

                    return run

                def opass_unit(u, heads, first):
                    t, nb = divmod(u, NB)

                    def run():
                        ps = yp.tile([128, 512], f32, tag="y")
                        for i, h in enumerate(heads):
                            nc.tensor.matmul(
                                ps[:],
                                outT_sb[:, h, 128 * t : 128 * (t + 1)],
                                wo_sb[:, h, 512 * nb : 512 * (nb + 1)],
                                start=(i == 0),
                                stop=(i == len(heads) - 1),
                            )
                        if first:
                            nc.vector.tensor_copy(part_sb[:, u, :], ps[:])
                        else:
                            nc.vector.tensor_add(
                                part_sb[:, u, :], part_sb[:, u, :], ps[:]
                            )

                    return run

                for qh in range(1, NH):
                    for sqh in range(2):
                        cell = {}
                        for cg in range(4):
                            unit_q.append(
                                ("q", qh, cg, q_subunit(qh, sqh, cg, cell))
                            )

                def close_open_stream():
                    # a partially-pumped Q stream holds a y-pool PSUM slot;
                    # emitting the head tail's rs/bc tiles against that slot
                    # before the stream's remaining matmuls would deadlock
                    while unit_q and unit_q[0][0] == "q" and unit_q[0][2] != 0:
                        unit_q.pop(0)[-1]()

                # bridge the gap between the end of Q0 and the arrival of the
                # gathered K/V (collective + readback) with early Q streams
                pump(12)

                for h in range(NH):
                    drain_q(h)
                    av = avp.tile([128, SQ], f32, tag="av")
                    eacc = eaccp.tile([128, SQ], bf16, tag="eacc")
                    e_tiles = {}

                    def emit_scores(c):
                        # scores^T chunk [Sk 128, Sq 1024] = (K^T slice)^T Q^T
                        sc = scp.tile([128, SQ], f32, tag="sc")
                        for sqh in range(2):
                            nc.tensor.matmul(
                                sc[:, 512 * sqh : 512 * (sqh + 1)],
                                kT_sb[:, 128 * c : 128 * (c + 1)],
                                qT_sb[:, h, 512 * sqh : 512 * (sqh + 1)],
                                start=True,
                                stop=True,
                            )
                        et = ep.tile([128, SQ], bf16, tag="e")
                        nc.scalar.activation(
                            et[:], sc[:], Exp,
                            bias=bias_sb[:, c : c + 1], scale=SCALE,
                        )
                        e_tiles[c] = et

                    def emit_av_acc(c):
                        et = e_tiles[c]
                        for sqh in range(2):
                            nc.tensor.matmul(
                                av[:, 512 * sqh : 512 * (sqh + 1)],
                                v_sb[:, c, :],
                                et[:, 512 * sqh : 512 * (sqh + 1)],
                                start=(c == 0),
                                stop=(c == SKC - 1),
                            )
                        # denominator accumulation stays off the PE; the last
                        # chunk is folded into the reduction matmul instead
                        if c == 0:
                            nc.vector.tensor_copy(eacc[:], et[:])
                        elif c < SKC - 1:
                            nc.vector.tensor_add(eacc[:], eacc[:], et[:])

                    emit_scores(0)
                    for c in range(SKC):
                        if c + 1 < SKC:
                            emit_scores(c + 1)
                        emit_av_acc(c)
                        # pump(2) only when the backlog won't otherwise fit in
                        # this head's remaining iterations; h5 stays at 1 so
                        # its pass-2 merges never backlog the DVE behind the
                        # eacc chain (leftovers drain into the tail chain)
                        if h == 4 and len(unit_q) > (SKC - 1 - c):
                            pump(2)
                        else:
                            pump(1)

                    # head tail: ones-matmul partition reduction of the
                    # denominator (+ last exp chunk), rank-1 broadcast,
                    # reciprocal + normalize. ACT copies av out of PSUM so
                    # the next head's AV accumulation isn't gated on this.
                    close_open_stream()
                    e15 = e_tiles[SKC - 1]
                    av_f = avsbp.tile([128, SQ], f32, tag="avf")
                    nc.scalar.activation(av_f[:], av[:], Copy)
                    for sqh in range(2):
                        sl = slice(512 * sqh, 512 * (sqh + 1))
                        rs = yp.tile([128, 512], f32, tag="y")
                        nc.tensor.matmul(
                            rs[0:1, :], ones_col[:], eacc[:, sl],
                            start=True, stop=False,
                        )
                        nc.tensor.matmul(
                            rs[0:1, :], ones_col[:], e15[:, sl],
                            start=False, stop=True,
                        )
                        row = rowp.tile([1, 512], bf16, tag="row")
                        nc.vector.tensor_copy(row[:], rs[0:1, :])
                        bc = yp.tile([128, 512], f32, tag="y")
                        nc.tensor.matmul(
                            bc[:], ones_row[:], row[:], start=True, stop=True
                        )
                        brec = brecp.tile([128, 512], f32, tag="brec")
                        nc.vector.reciprocal_approx_fast(brec[:], bc[:])
                        nc.vector.tensor_mul(
                            outT_sb[:, h, sl], av_f[:, sl], brec[:]
                        )

                    if h == 2:
                        for u in range(NU):
                            unit_q.append(("o", opass_unit(u, (0, 1, 2), True)))
                    elif h == 4:
                        for u in range(NU):
                            unit_q.append(("o", opass_unit(u, (3, 4), False)))
                    # keep the PE fed while the normalization chain resolves
                    pump(2)

                # drain leftovers, then the final o-pass: bf16 partial folded
                # back into PSUM via an identity matmul + h5's contribution.
                # The sc/av pools are dead now, so pass-3 tiles spread over
                # all 8 PSUM banks (8 units in flight) and writebacks
                # alternate ACT/DVE so neither engine serializes the drain.
                pump(len(unit_q))

                p3_cache = {}

                def p3_tile(u):
                    k = u % 4
                    if k == 0:
                        t = scp.tile([128, SQ], f32, tag="sc", name=f"p3s{u}")
                        p3_cache["t"] = t
                        return t[:, 0:512]
                    if k == 1:
                        return p3_cache["t"][:, 512:]
                    if k == 2:
                        t = avp.tile([128, SQ], f32, tag="av", name=f"p3a{u}")
                        p3_cache["t"] = t
                        return t[:, 0:512]
                    return p3_cache["t"][:, 512:]

                for t in range(NT):
                    ysb_t = ysb.tile([128, NB, 512], bf16, tag="ysb")
                    for nb in range(NB):
                        u = t * NB + nb
                        ps = p3_tile(u)
                        nc.tensor.matmul(
                            ps, ident[:], part_sb[:, u, :], start=True, stop=False
                        )
                        nc.tensor.matmul(
                            ps,
                            outT_sb[:, 5, 128 * t : 128 * (t + 1)],
                            wo_sb[:, 5, 512 * nb : 512 * (nb + 1)],
                            start=False,
                            stop=True,
                        )
                        if u % 2 == 0:
                            nc.scalar.activation(ysb_t[:, nb, :], ps, Copy)
                        else:
                            nc.vector.tensor_copy(ysb_t[:, nb, :], ps)
                    nc.sync.dma_start(y[128 * t : 128 * (t + 1), :], ysb_t[:])

    nc.compile()
    return nc


def _get_nc():
    if "nc" not in _NC_CACHE:
        _NC_CACHE["nc"] = _build_nc()
    return _NC_CACHE["nc"]


def kernel(hidden_states, attention_mask, Wq, Wk, Wv, Wo):
    global last_results
    from concourse.bass_utils import run_bass_kernel_spmd

    bf = ml_dtypes.bfloat16
    hidden_states = np.asarray(hidden_states, dtype=np.float32)
    attention_mask = np.asarray(attention_mask, dtype=np.float32)
    Wq = np.asarray(Wq, dtype=np.float32)
    Wk = np.asarray(Wk, dtype=np.float32)
    Wv = np.asarray(Wv, dtype=np.float32)
    Wo = np.asarray(Wo, dtype=np.float32)

    nc = _get_nc()

    def part_major(w, rows):
        # [rows*128, m] -> [128, rows, m] (partition-major, contiguous lines)
        return np.ascontiguousarray(
            w.reshape(rows, 128, w.shape[-1]).transpose(1, 0, 2)
        )

    in_maps = []
    cores = []
    for b in range(2):
        xt_full = hidden_states[b].T.astype(bf)  # [H, S]
        bias_full = ((1.0 - attention_mask[b]) * -10000.0).astype(np.float32)
        for hh in range(2):
            wqt = part_major(
                np.ascontiguousarray(Wq[NH * D * hh : NH * D * (hh + 1), :].T)
                .astype(bf), HC)
            wkt = part_major(
                np.ascontiguousarray(Wk[D * hh : D * (hh + 1), :].T).astype(bf), HC)
            wvt = part_major(
                np.ascontiguousarray(Wv[D * hh : D * (hh + 1), :].T).astype(bf), HC)
            wot = part_major(
                np.ascontiguousarray(Wo[:, NH * D * hh : NH * D * (hh + 1)].T)
                .astype(bf), NH)
            # bias in GLOBAL key order (kT_sb/v_sb are gathered globally)
            biasd = np.ascontiguousarray(
                bias_full.reshape(SKC, 128).T
            ).astype(np.float32)
            for th in range(2):
                # this core's query tokens (also its K/V projection shard)
                xt_r = part_major(xt_full[:, th * SQ : (th + 1) * SQ], HC)
                in_maps.append(
                    {
                        "xt": xt_r,
                        "wqt": wqt,
                        "wkt": wkt,
                        "wvt": wvt,
                        "wot": wot,
                        "biasd": biasd,
                    }
                )
                cores.append((b, hh, th))

    res = run_bass_kernel_spmd(nc, in_maps, core_ids=list(range(8)))
    last_results = res

    out = np.zeros((B, S, HIDDEN), dtype=np.float32)
    for (b, hh, th), r in zip(cores, res.results):
        out[b, th * SQ : (th + 1) * SQ, :] += np.asarray(r["y"], dtype=np.float32)
    return out



# revision 11
# speedup vs baseline: 1.2081x; 1.2081x over previous
"""DeepSeek-style GQA attention block (B=2, S=2048, H=1536, 12 q-heads /
2 kv-heads, d=128) sharded over 8 TRN2 NeuronCores.

Sharding: core = (batch b, kv-group hh, query-half th).
  - tensor parallel over the 2 kv groups (6 q-heads + 1 kv head each)
  - data parallel over batch (2)
  - query-token parallel (2 halves of 1024)
Each core computes its 6 heads' attention for its 1024 query tokens against
the full 2048-token K/V of its kv head, then a partial O-projection; the two
kv-group partials per (b, th) are summed on the host.

All matmuls run in bf16 with fp32 PSUM accumulation. Softmax runs without
max-subtraction (scores are O(1) here), with the 1/sqrt(d) scale and the
additive attention-mask bias fused into the ACT exp instruction.

Layout trick: scores are computed TRANSPOSED (scores^T[Sk, Sq] = K Q^T) so
the attention probabilities come out with Sk on partitions, which is exactly
the layout the AV matmul needs as its moving operand.

Softmax denominator: exp chunks are accumulated on the DVE (bf16 adds, with
the last chunk folded directly into the reduction matmul), then ONE ones-
vector matmul per head-half does the 128-partition reduction, a rank-1
matmul broadcasts it back over the d partitions, and DVE reciprocal/multiply
normalizes the AV output. This costs the PE ~1.3us/head instead of the
~8us/head of a per-chunk rowsum scheme, and keeps the slow GpSimd engine
(partition_all_reduce measured 6.7us!) off the critical path entirely.

Scheduling: TRN2's PE p-state throttle only reaches 2.4 GHz after 3us of
uninterrupted work, so the kernel keeps the PE queue dense. All deferrable
matmul work — Q-projection for heads 1-5 and the first two O-projection
passes (h0-2, h3-4, merged through a bf16 SBUF partial) — is enqueued as
closures and pumped one unit per chunk-iteration inside the attention
loops, filling what would otherwise be exp-wait bubbles. After head 5 only
a single-matmul O-pass remains: partial + h5 contribution accumulate in
PSUM via an identity matmul, with writebacks alternating ACT/DVE.
"""

import numpy as np
import ml_dtypes

HIDDEN = 1536
D = 128          # head dim
NH = 6           # q-heads per core
B, S = 2, 2048
SQ = 1024        # query tokens per core
HC = HIDDEN // 128   # 12 hidden chunks
SKC = S // 128       # 16 key chunks
SCALE = float(1.0 / np.sqrt(np.float32(D)))

_NC_CACHE = {}
last_results = None  # BassKernelResults of the most recent run (for test.py)


def _build_nc():
    import concourse.bacc as bacc
    import concourse.mybir as mybir
    import concourse.tile as tile
    from concourse.masks import make_identity

    bf16 = mybir.dt.bfloat16
    f32 = mybir.dt.float32
    Exp = mybir.ActivationFunctionType.Exp
    Copy = mybir.ActivationFunctionType.Copy

    nc = bacc.Bacc("TRN2", target_bir_lowering=False, debug=False, num_devices=8)

    # All inputs are pre-rearranged on the host into [128-partition, ...]
    # layouts so every DMA line is a large contiguous run (few descriptors).
    # xt holds only this core's 1024 query tokens: K/V for the other half
    # comes from the partner core via a pairwise AllGather.
    xt = nc.dram_tensor("xt", [128, HC, SQ], bf16, kind="ExternalInput")
    wqt = nc.dram_tensor("wqt", [128, HC, NH * D], bf16, kind="ExternalInput")
    wkt = nc.dram_tensor("wkt", [128, HC, D], bf16, kind="ExternalInput")
    wvt = nc.dram_tensor("wvt", [128, HC, D], bf16, kind="ExternalInput")
    wot = nc.dram_tensor("wot", [128, NH, HIDDEN], bf16, kind="ExternalInput")
    biasd = nc.dram_tensor("biasd", [128, SKC], f32, kind="ExternalInput")
    y = nc.dram_tensor("y", [SQ, HIDDEN], bf16, kind="ExternalOutput")

    NB = HIDDEN // 512   # 3 o-proj column blocks
    NT = SQ // 128       # 8 o-proj token blocks
    NU = NT * NB         # 24 o-proj tiles

    with tile.TileContext(nc) as tc:
        with (
            tc.tile_pool(name="const", bufs=1) as constp,
            tc.tile_pool(name="weights", bufs=1) as wp,
            tc.tile_pool(name="persist", bufs=1) as pers,
        ):
            ident = constp.tile([128, 128], bf16)
            make_identity(nc, ident[:])
            ones_col = constp.tile([128, 1], bf16)
            nc.vector.memset(ones_col[:], 1.0)
            ones_row = constp.tile([1, 128], bf16)
            nc.vector.memset(ones_row[:], 1.0)
            bias_sb = constp.tile([128, SKC], f32)

            wk_sb = wp.tile([128, HC, D], bf16)
            wv_sb = wp.tile([128, HC, D], bf16)
            wo_sb = wp.tile([128, NH, HIDDEN], bf16)

            kT_sb = pers.tile([128, S], bf16)         # K^T [d, Sk] (global order)
            v_sb = pers.tile([128, SKC, D], bf16)     # V [Sk, d], chunked
            qT_sb = pers.tile([128, NH, SQ], bf16)    # Q^T [d, Sq] per head
            outT_sb = pers.tile([128, NH, SQ], bf16)  # normalized AV out^T
            part_sb = pers.tile([128, NU, 512], bf16) # o-proj partials
            xt_sb = pers.tile([128, HC, SQ], bf16)
            wq_sb = pers.tile([128, HC, NH * D], bf16)
            vT_sb = pers.tile([128, SQ], bf16)
            ccs_sb = pers.tile([128, 2048], bf16)     # [kT half | v half] staging

            def dma_psplit(dst, src, n):
                # split a [128, ...] DMA into n partition ranges so the
                # per-partition-line descriptors issue on n queues in parallel
                step = 128 // n
                for i in range(n):
                    sl = slice(step * i, step * (i + 1))
                    nc.sync.dma_start(dst[sl], src[sl])

            # DMA order is the early-pipeline schedule. Each transfer moves
            # whole-tensor per-partition contiguous lines (descriptor cost is
            # ~constant per line, so bigger lines = more GB/s per queue),
            # partition-split across queues for parallelism. wo is issued
            # LAST (after the collective's bounce traffic) since o-proj
            # weights aren't needed until after head 2.
            dma_psplit(wk_sb[:], wkt.ap(), 4)
            dma_psplit(wv_sb[:], wvt.ap(), 4)
            dma_psplit(xt_sb[:, 0, :], xt[:, 0, :], 2)
            dma_psplit(xt_sb[:, 1, :], xt[:, 1, :], 2)
            for c in range(2, HC):
                nc.sync.dma_start(xt_sb[:, c, :], xt[:, c, :])
            nc.sync.dma_start(bias_sb[:], biasd.ap())
            for cq in range(0, HC, 2):
                nc.sync.dma_start(
                    wq_sb[:, cq : cq + 2, :], wqt[:, cq : cq + 2, :]
                )

            # ---------- local K/V half + exchange, Q0 projection ----------
            # Each core projects K/V only for its own 1024 tokens; the two
            # query-half cores of the same (batch, kv-group) swap halves via
            # a pairwise AllGather through DRAM bounce buffers. kT_sb / v_sb
            # end up in GLOBAL token order on both cores.
            with (
                tc.tile_pool(name="kv_ps", bufs=1, space="PSUM") as kvps,
                tc.tile_pool(name="proj_ps", bufs=1, space="PSUM") as pps,
                tc.tile_pool(name="vtr_ps", bufs=2, space="PSUM") as vtrp,
                tc.tile_pool(name="cc_dram", bufs=1, space="DRAM") as ccd,
            ):
                ccin = ccd.tile([128, 2048], bf16)
                ccout = ccd.tile([2, 128, 2048], bf16)
                qps = pps.tile([128, SQ], f32, tag="projq")

                kv = kvps.tile([128, 2, 2, 512], f32, tag="kv")
                for c in range(HC):
                    for ti, w_sb in ((0, wk_sb), (1, wv_sb)):
                        for sb in range(2):
                            nc.tensor.matmul(
                                kv[:, ti, sb, :],
                                w_sb[:, c, :],
                                xt_sb[:, c, 512 * sb : 512 * (sb + 1)],
                                start=(c == 0),
                                stop=(c == HC - 1),
                            )
                for sb in range(2):
                    nc.vector.tensor_copy(
                        vT_sb[:, 512 * sb : 512 * (sb + 1)], kv[:, 1, sb, :]
                    )
                for sb in range(2):
                    nc.vector.tensor_copy(
                        ccs_sb[:, 512 * sb : 512 * (sb + 1)], kv[:, 0, sb, :]
                    )
                for c in range(8):
                    pt = vtrp.tile([128, 128], bf16, tag="vtr")
                    nc.tensor.transpose(
                        pt[:], vT_sb[:, 128 * c : 128 * (c + 1)], ident[:]
                    )
                    nc.vector.tensor_copy(
                        ccs_sb[:, 1024 + 128 * c : 1024 + 128 * (c + 1)], pt[:]
                    )
                dma_psplit(ccin[:, :], ccs_sb[:, :], 4)
                nc.gpsimd.collective_compute(
                    "AllGather",
                    mybir.AluOpType.bypass,
                    replica_groups=[[0, 1], [2, 3], [4, 5], [6, 7]],
                    ins=[ccin.opt()],
                    outs=[ccout.opt()],
                )
                for sqh in range(2):
                    for c in range(HC):
                        nc.tensor.matmul(
                            qps[:, 512 * sqh : 512 * (sqh + 1)],
                            wq_sb[:, c, 0:D],
                            xt_sb[:, c, 512 * sqh : 512 * (sqh + 1)],
                            start=(c == 0),
                            stop=(c == HC - 1),
                        )
                nc.vector.tensor_copy(qT_sb[:, 0, :], qps[:])
                for th in range(2):
                    dma_psplit(
                        kT_sb[:, 1024 * th : 1024 * (th + 1)],
                        ccout[th, :, 0:1024], 2,
                    )
                    dma_psplit(
                        v_sb[:, 8 * th : 8 * (th + 1), :],
                        ccout[th, :, 1024:2048], 2,
                    )
                dma_psplit(wo_sb[:], wot.ap(), 8)

            # ---------- attention + pipelined projections ----------
            with (
                tc.tile_pool(name="sc_ps", bufs=2, space="PSUM") as scp,
                tc.tile_pool(name="av_ps", bufs=1, space="PSUM") as avp,
                tc.tile_pool(name="y_ps", bufs=2, space="PSUM") as yp,
                tc.tile_pool(name="esb", bufs=8) as ep,
                tc.tile_pool(name="eaccp", bufs=2) as eaccp,
                tc.tile_pool(name="rowp", bufs=4) as rowp,
                tc.tile_pool(name="brecp", bufs=2) as brecp,
                tc.tile_pool(name="avsb", bufs=2) as avsbp,
                tc.tile_pool(name="y_sb", bufs=6) as ysb,
            ):
                # deferred PE work: ('q', head, fn) or ('o', fn)
                unit_q = []

                def pump(n):
                    for _ in range(min(n, len(unit_q))):
                        unit_q.pop(0)[-1]()

                def drain_q(h):
                    while unit_q and unit_q[0][0] == "q" and unit_q[0][1] <= h:
                        unit_q.pop(0)[-1]()

                def q_subunit(qh, sqh, cg, cell):
                    # 3-matmul slice of one Q head-half projection; the four
                    # slices share a PSUM tile carried through `cell`
                    def run():
                        if cg == 0:
                            cell["ps"] = yp.tile(
                                [128, 512], f32, tag="y", name=f"qp{qh}_{sqh}"
                            )
                        ps = cell["ps"]
                        for c in range(3 * cg, 3 * cg + 3):
                            nc.tensor.matmul(
                                ps[:],
                                wq_sb[:, c, D * qh : D * (qh + 1)],
                                xt_sb[:, c, 512 * sqh : 512 * (sqh + 1)],
                                start=(c == 0),
                                stop=(c == HC - 1),
                            )
                        if cg == 3:
                            nc.vector.tensor_copy(
                                qT_sb[:, qh, 512 * sqh : 512 * (sqh + 1)], ps[:]
                            )

                    return run

                def opass_unit(u, heads, first):
                    t, nb = divmod(u, NB)

                    def run():
                        ps = yp.tile([128, 512], f32, tag="y")
                        for i, h in enumerate(heads):
                            nc.tensor.matmul(
                                ps[:],
                                outT_sb[:, h, 128 * t : 128 * (t + 1)],
                                wo_sb[:, h, 512 * nb : 512 * (nb + 1)],
                                start=(i == 0),
                                stop=(i == len(heads) - 1),
                            )
                        if first:
                            nc.vector.tensor_copy(part_sb[:, u, :], ps[:])
                        else:
                            nc.vector.tensor_add(
                                part_sb[:, u, :], part_sb[:, u, :], ps[:]
                            )

                    return run

                for qh in range(1, NH):
                    for sqh in range(2):
                        cell = {}
                        for cg in range(4):
                            unit_q.append(
                                ("q", qh, cg, q_subunit(qh, sqh, cg, cell))
                            )

                def close_open_stream():
                    # a partially-pumped Q stream holds a y-pool PSUM slot;
                    # emitting the head tail's rs/bc tiles against that slot
                    # before the stream's remaining matmuls would deadlock
                    while unit_q and unit_q[0][0] == "q" and unit_q[0][2] != 0:
                        unit_q.pop(0)[-1]()

                # bridge the gap between the end of Q0 and the arrival of the
                # gathered K/V (collective + readback) with early Q streams
                pump(12)

                for h in range(NH):
                    drain_q(h)
                    av = avp.tile([128, SQ], f32, tag="av")
                    eacc = eaccp.tile([128, SQ], bf16, tag="eacc")
                    e_tiles = {}

                    def emit_scores(c):
                        # scores^T chunk [Sk 128, Sq 1024] = (K^T slice)^T Q^T
                        sc = scp.tile([128, SQ], f32, tag="sc")
                        for sqh in range(2):
                            nc.tensor.matmul(
                                sc[:, 512 * sqh : 512 * (sqh + 1)],
                                kT_sb[:, 128 * c : 128 * (c + 1)],
                                qT_sb[:, h, 512 * sqh : 512 * (sqh + 1)],
                                start=True,
                                stop=True,
                            )
                        et = ep.tile([128, SQ], bf16, tag="e")
                        nc.scalar.activation(
                            et[:], sc[:], Exp,
                            bias=bias_sb[:, c : c + 1], scale=SCALE,
                        )
                        e_tiles[c] = et

                    def emit_av_acc(c):
                        et = e_tiles[c]
                        for sqh in range(2):
                            nc.tensor.matmul(
                                av[:, 512 * sqh : 512 * (sqh + 1)],
                                v_sb[:, c, :],
                                et[:, 512 * sqh : 512 * (sqh + 1)],
                                start=(c == 0),
                                stop=(c == SKC - 1),
                            )
                        # denominator accumulation stays off the PE; the last
                        # chunk is folded into the reduction matmul instead
                        if c == 0:
                            nc.vector.tensor_copy(eacc[:], et[:])
                        elif c < SKC - 1:
                            nc.vector.tensor_add(eacc[:], eacc[:], et[:])

                    emit_scores(0)
                    for c in range(SKC):
                        if c + 1 < SKC:
                            emit_scores(c + 1)
                        emit_av_acc(c)
                        # pump(2) only when the backlog won't otherwise fit in
                        # this head's remaining iterations; h5 stays at 1 so
                        # its pass-2 merges never backlog the DVE behind the
                        # eacc chain (leftovers drain into the tail chain)
                        if h == 4 and len(unit_q) > (SKC - 1 - c):
                            pump(2)
                        else:
                            pump(1)

                    # head tail: ones-matmul partition reduction of the
                    # denominator (+ last exp chunk), rank-1 broadcast,
                    # reciprocal + normalize. ACT copies av out of PSUM so
                    # the next head's AV accumulation isn't gated on this.
                    close_open_stream()
                    e15 = e_tiles[SKC - 1]
                    av_f = avsbp.tile([128, SQ], f32, tag="avf")
                    nc.scalar.activation(av_f[:], av[:], Copy)
                    for sqh in range(2):
                        sl = slice(512 * sqh, 512 * (sqh + 1))
                        rs = yp.tile([128, 512], f32, tag="y")
                        nc.tensor.matmul(
                            rs[0:1, :], ones_col[:], eacc[:, sl],
                            start=True, stop=False,
                        )
                        nc.tensor.matmul(
                            rs[0:1, :], ones_col[:], e15[:, sl],
                            start=False, stop=True,
                        )
                        row = rowp.tile([1, 512], bf16, tag="row")
                        nc.vector.tensor_copy(row[:], rs[0:1, :])
                        bc = yp.tile([128, 512], f32, tag="y")
                        nc.tensor.matmul(
                            bc[:], ones_row[:], row[:], start=True, stop=True
                        )
                        brec = brecp.tile([128, 512], f32, tag="brec")
                        nc.vector.reciprocal_approx_fast(brec[:], bc[:])
                        nc.vector.tensor_mul(
                            outT_sb[:, h, sl], av_f[:, sl], brec[:]
                        )

                    if h == 2:
                        for u in range(NU):
                            unit_q.append(("o", opass_unit(u, (0, 1, 2), True)))
                    elif h == 4:
                        for u in range(NU):
                            unit_q.append(("o", opass_unit(u, (3, 4), False)))
                    # keep the PE fed while the normalization chain resolves
                    pump(2)

                # drain leftovers, then the final o-pass: bf16 partial folded
                # back into PSUM via an identity matmul + h5's contribution.
                # The sc/av pools are dead now, so pass-3 tiles spread over
                # all 8 PSUM banks (8 units in flight) and writebacks
                # alternate ACT/DVE so neither engine serializes the drain.
                pump(len(unit_q))

                p3_cache = {}

                def p3_tile(u):
                    k = u % 4
                    if k == 0:
                        t = scp.tile([128, SQ], f32, tag="sc", name=f"p3s{u}")
                        p3_cache["t"] = t
                        return t[:, 0:512]
                    if k == 1:
                        return p3_cache["t"][:, 512:]
                    if k == 2:
                        t = avp.tile([128, SQ], f32, tag="av", name=f"p3a{u}")
                        p3_cache["t"] = t
                        return t[:, 0:512]
                    return p3_cache["t"][:, 512:]

                for t in range(NT):
                    ysb_t = ysb.tile([128, NB, 512], bf16, tag="ysb")
                    for nb in range(NB):
                        u = t * NB + nb
                        ps = p3_tile(u)
                        nc.tensor.matmul(
                            ps, ident[:], part_sb[:, u, :], start=True, stop=False
                        )
                        nc.tensor.matmul(
                            ps,
                            outT_sb[:, 5, 128 * t : 128 * (t + 1)],
                            wo_sb[:, 5, 512 * nb : 512 * (nb + 1)],
                            start=False,
                            stop=True,
                        )
                        if u % 2 == 0:
                            nc.scalar.activation(ysb_t[:, nb, :], ps, Copy)
                        else:
                            nc.vector.tensor_copy(ysb_t[:, nb, :], ps)
                    nc.sync.dma_start(y[128 * t : 128 * (t + 1), :], ysb_t[:])

    nc.compile()
    return nc


def _get_nc():
    if "nc" not in _NC_CACHE:
        _NC_CACHE["nc"] = _build_nc()
    return _NC_CACHE["nc"]


def kernel(hidden_states, attention_mask, Wq, Wk, Wv, Wo):
    global last_results
    from concourse.bass_utils import run_bass_kernel_spmd

    bf = ml_dtypes.bfloat16
    hidden_states = np.asarray(hidden_states, dtype=np.float32)
    attention_mask = np.asarray(attention_mask, dtype=np.float32)
    Wq = np.asarray(Wq, dtype=np.float32)
    Wk = np.asarray(Wk, dtype=np.float32)
    Wv = np.asarray(Wv, dtype=np.float32)
    Wo = np.asarray(Wo, dtype=np.float32)

    nc = _get_nc()

    def part_major(w, rows):
        # [rows*128, m] -> [128, rows, m] (partition-major, contiguous lines)
        return np.ascontiguousarray(
            w.reshape(rows, 128, w.shape[-1]).transpose(1, 0, 2)
        )

    in_maps = []
    cores = []
    for b in range(2):
        xt_full = hidden_states[b].T.astype(bf)  # [H, S]
        bias_full = ((1.0 - attention_mask[b]) * -10000.0).astype(np.float32)
        for hh in range(2):
            wqt = part_major(
                np.ascontiguousarray(Wq[NH * D * hh : NH * D * (hh + 1), :].T)
                .astype(bf), HC)
            wkt = part_major(
                np.ascontiguousarray(Wk[D * hh : D * (hh + 1), :].T).astype(bf), HC)
            wvt = part_major(
                np.ascontiguousarray(Wv[D * hh : D * (hh + 1), :].T).astype(bf), HC)
            wot = part_major(
                np.ascontiguousarray(Wo[:, NH * D * hh : NH * D * (hh + 1)].T)
                .astype(bf), NH)
            # bias in GLOBAL key order (kT_sb/v_sb are gathered globally)
            biasd = np.ascontiguousarray(
                bias_full.reshape(SKC, 128).T
            ).astype(np.float32)
            for th in range(2):
                # this core's query tokens (also its K/V projection shard)
                xt_r = part_major(xt_full[:, th * SQ : (th + 1) * SQ], HC)
                in_maps.append(
                    {
                        "xt": xt_r,
                        "wqt": wqt,
                        "wkt": wkt,
                        "wvt": wvt,
                        "wot": wot,
                        "biasd": biasd,
                    }
                )
                cores.append((b, hh, th))

    res = run_bass_kernel_spmd(nc, in_maps, core_ids=list(range(8)))
    last_results = res

    out = np.zeros((B, S, HIDDEN), dtype=np.float32)
    for (b, hh, th), r in zip(cores, res.results):
        out[b, th * SQ : (th + 1) * SQ, :] += np.asarray(r["y"], dtype=np.float32)
    return out

